# revision 29
# baseline (speedup 1.0000x reference)
"""Trainium2 Bass kernel for nn_CellLayer (GRU over B=16, T=4096, D=256, H=512).

Strategy: chunk-parallel GRU with warmup, in a TRANSPOSED layout:
  - T=4096 split into C=128 chunks of L=32 steps; 16 chunks/core x 16 batch
    = 256 lanes per core, stepped S = L + V slots (V=10 warmup).
  - Chunk 0 is time-shifted (starts exactly at t=0 from h=0, which is exact),
    so no masking is needed anywhere; all S slots are written out and the host
    picks each chunk's valid window.
  - Layout: gates and hidden state are [h-dim (partition), lane (free)].
    Benefits vs the [lane, h-dim] layout:
      * h' is produced directly in the stationary operand layout for the next
        step's W_hh matmul -> no PE transposes, no hT copies.
      * All biases are per-partition -> fused for free into ACT activations
        (sigmoid/tanh bias) and one DVE scalar_tensor_tensor (b_n).
      * With 256 lanes the matmul moving dim is 256 >= 256, so float32r runs
        at 1 cycle/row (full PE speed), same as bf16, with fp32-grade accuracy.
  - Per step: 48 h-side + 24 x-side matmuls of [128c x 128m] x [128c, 256]
    accumulating into 8 PSUM banks (4x prz = r|z pairs, 4x pnn = ni|nh pairs).
    x-matmuls for step s+1 are interleaved mid-stream so the PE never idles.
  - Elementwise gate math split across ACT (r, z, n), DVE (t2, t3, m) and
    GPSIMD/Pool (d, h') so no engine exceeds ~70% and PE stays the bottleneck.
"""

import os
import sys

sys.path.insert(0, "/opt/trn_rl_repo")

import numpy as np

import concourse.bass as bass
import concourse.mybir as mybir
import concourse.tile as tile
from concourse import bacc
from concourse.bass import ds, ts
from concourse.bass_utils import run_bass_kernel_spmd

B, T, D, H = 16, 4096, 256, 512
G = 3 * H
NCORES = 8
CPC = 16  # chunks per core
C = NCORES * CPC  # 128 chunks
L = T // C  # 32 output steps per chunk
V = 10  # warmup steps (fp32 chunked algo: l2 rel err 7.2e-4, max-rel 7.0e-3)
S = L + V  # 42 slots
if os.environ.get("KERNEL_S_OVERRIDE"):  # dev: truncated build for fast iteration
    S = int(os.environ["KERNEL_S_OVERRIDE"])
LAN = CPC * B  # 256 lanes = (chunk_local, batch)
P = 128
DK = D // P  # 2 x-contract chunks
HK = H // P  # 4 h-contract chunks / h subtiles

F32 = mybir.dt.float32
F32R = mybir.dt.float32r

_cached = {}


def build_nc():
    nc = bacc.Bacc(None, target_bir_lowering=False)

    # ---- DRAM I/O (per-core values supplied via in_maps) ----
    # xs_t[s, d, lane]: x for slot s, d-major (zeros for chunk0's tail slots)
    xs_t = nc.declare_dram_parameter("xs_t", [S, D, LAN], F32R, isOutput=False)
    # weights, pre-transposed on host: w_hh_t[h, g], w_ih_t[d, g]
    w_hh_t = nc.declare_dram_parameter("w_hh_t", [H, G], F32R, isOutput=False)
    w_ih_t = nc.declare_dram_parameter("w_ih_t", [D, G], F32R, isOutput=False)
    # bias columns [p, 16]: cols 0-3 b_r_j, 4-7 b_z_j, 8-11 b_in_j, 12-15 b_n_j
    bcol = nc.declare_dram_parameter("bcol", [P, 16], F32, isOutput=False)
    # output: ys[s, h, lane] for ALL slots (host selects valid windows)
    ys = nc.declare_dram_parameter("ys", [S, H, LAN], F32R, isOutput=True)

    with tile.TileContext(nc) as tc:
        _build_body(nc, tc, xs_t, w_hh_t, w_ih_t, bcol, ys)
    nc.compile()
    return nc


def _build_body(nc, tc, xs_t, w_hh_t, w_ih_t, bcol, ys):
    from contextlib import ExitStack

    add = mybir.AluOpType.add
    sub = mybir.AluOpType.subtract
    mult = mybir.AluOpType.mult
    SIG = mybir.ActivationFunctionType.Sigmoid
    TANH = mybir.ActivationFunctionType.Tanh

    def gsl(g, j):  # weight columns of gate g, h-subtile j
        return ds(g * H + j * P, P)

    ctx = ExitStack()
    with ctx:
        const = ctx.enter_context(tc.tile_pool(name="const", bufs=1))
        xpool = ctx.enter_context(tc.tile_pool(name="xpool", bufs=4))
        hpool = ctx.enter_context(tc.tile_pool(name="hpool", bufs=2))
        gates = ctx.enter_context(tc.tile_pool(name="gates", bufs=2))
        psum = ctx.enter_context(tc.tile_pool(name="psum", bufs=1, space="PSUM"))

        # ---- resident constants ----
        # order matters: bc/wih feed step 0 (s0 skips h-matmuls), so the big
        # W_hh transfer goes last, split per contract-chunk, overlapping s=0.
        bc = const.tile([P, 16], F32)
        nc.sync.dma_start(bc[:], bcol[:])
        wih = const.tile([P, DK, G], F32R)
        nc.sync.dma_start(wih[:], w_ih_t.rearrange("(dk p) g -> p dk g", p=P))
        whh = const.tile([P, HK, G], F32R)  # [h%128, h//128, g]

        # ---- h state: 4 subtiles [h%128, lane], ring of 2 each ----
        hcur = []
        for j in range(HK):
            hj = hpool.tile([P, LAN], F32R, name=f"hn{j}")
            nc.vector.memset(hj[:].bitcast(F32), 0.0)
            hcur.append(hj)

# PSUM bank discipline: a matmul with start=True clears the whole bank's
        # has-written bits (data survives, but another group's in-progress
        # accumulation breaks). So within one bank, a group's [first..last]
        # write window must contain no other group's start.
        #   bank A_j = r_j | z_j:  z's group runs strictly after r's stop.
        #   bank B_j = ni_j | nh_j: ni (x-only) closes in step s-1; nh after.

        def new_A():
            return [psum.tile([P, 2 * LAN], F32, name=f"pA{j}") for j in range(HK)]

        def new_B():
            return [psum.tile([P, 2 * LAN], F32, name=f"pB{j}") for j in range(HK)]

        def emit_xr(A, xt, j, s0=False):  # open r window (s0: h==0, close it too)
            for k in range(DK):
                nc.tensor.matmul(A[j][:, 0:LAN], wih[:, k, gsl(0, j)], xt[:, k], start=(k == 0), stop=(s0 and k == DK - 1))

        def emit_xni(B, xt, j):  # ni: x-only, complete group
            for k in range(DK):
                nc.tensor.matmul(B[j][:, 0:LAN], wih[:, k, gsl(2, j)], xt[:, k], start=(k == 0), stop=(k == DK - 1))

        def emit_z_block(A, xt, hsrc, j, s0=False):  # full z group (after r's stop)
            for k in range(DK):
                nc.tensor.matmul(A[j][:, LAN:], wih[:, k, gsl(1, j)], xt[:, k], start=(k == 0), stop=(s0 and k == DK - 1))
            if not s0:
                for k in range(HK):
                    nc.tensor.matmul(A[j][:, LAN:], whh[:, k, gsl(1, j)], hsrc[k][:], start=False, stop=(k == HK - 1))

        # ---- prologue: ni(0), xr(0), xt prefetch ----
        xt_cur = xpool.tile([P, DK, LAN], F32R, name="xt")
        nc.sync.dma_start(xt_cur[:], xs_t[0].rearrange("(dk p) b -> p dk b", p=P))
        xt_next = None
        if S > 1:
            xt_next = xpool.tile([P, DK, LAN], F32R, name="xt")
            nc.sync.dma_start(xt_next[:], xs_t[1].rearrange("(dk p) b -> p dk b", p=P))
        cur_B = new_B()
        for j in range(HK):
            emit_xni(cur_B, xt_cur, j)
            # s=0 skips the h-matmuls entirely (h==0): nh half must read 0
            nc.vector.memset(cur_B[j][:, LAN:], 0.0)
        # big W_hh load queued after everything step 0 needs; first use is s=1
        for k in range(HK):
            nc.sync.dma_start(whh[:, k, :], w_hh_t[ds(k * P, P), :])

        for s in range(S):
            last = s == S - 1
            if not last and s + 2 < S:
                xt_pre = xpool.tile([P, DK, LAN], F32R, name="xt")
                nc.sync.dma_start(xt_pre[:], xs_t[s + 2].rearrange("(dk p) b -> p dk b", p=P))
            else:
                xt_pre = None

            s0 = s == 0  # h==0: h-matmuls skipped; W_hh DMA overlaps step 0
            cur_A = new_A()
            # phi0: open all r windows (x-side, no h dependency)
            for j in range(HK):
                emit_xr(cur_A, xt_cur, j, s0)

            # per-j sets: [hr_j k0..3 | hnh_j k0..3 | z_j block], so each
            # bank's stops stagger early and the gate chain starts ~1.3us in
            # instead of at stream end. Chain tail: c = z*h runs off-chain at
            # z-time; e = (z-1)*n; h' = c - e (one serial stage fewer).
            hnew = []
            for j in range(HK):
                if not s0:
                    for k in range(HK):
                        nc.tensor.matmul(cur_A[j][:, 0:LAN], whh[:, k, gsl(0, j)], hcur[k][:], start=False, stop=(k == HK - 1))
                    for k in range(HK):
                        nc.tensor.matmul(cur_B[j][:, LAN:], whh[:, k, gsl(2, j)], hcur[k][:], start=(k == 0), stop=(k == HK - 1))
                emit_z_block(cur_A, xt_cur, hcur, j, s0)

                rj = gates.tile([P, LAN], F32, name=f"r{j}")
                nc.scalar.activation(rj[:], cur_A[j][:, 0:LAN], SIG, bias=bc[:, ds(j, 1)])
                zj = gates.tile([P, LAN], F32, name=f"z{j}")
                nc.scalar.activation(zj[:], cur_A[j][:, LAN:], SIG, bias=bc[:, ds(4 + j, 1)])
                cj = gates.tile([P, LAN], F32, name=f"c{j}")
                nc.gpsimd.tensor_tensor(cj[:], zj[:], hcur[j][:], mult)
                t2j = gates.tile([P, LAN], F32, name=f"t2{j}")
                nc.vector.scalar_tensor_tensor(t2j[:], cur_B[j][:, LAN:], bc[:, ds(12 + j, 1)], rj[:], add, mult)
                t3j = gates.tile([P, LAN], F32, name=f"t3{j}")
                nc.vector.tensor_tensor(t3j[:], t2j[:], cur_B[j][:, 0:LAN], add)
                nj = gates.tile([P, LAN], F32, name=f"n{j}")
                nc.scalar.activation(nj[:], t3j[:], TANH, bias=bc[:, ds(8 + j, 1)])
                ej = gates.tile([P, LAN], F32, name=f"e{j}")
                nc.vector.scalar_tensor_tensor(ej[:], zj[:], 1.0, nj[:], sub, mult)
                hj = hpool.tile([P, LAN], F32R, name=f"hn{j}")
                heng = nc.gpsimd if j < 2 else nc.vector
                heng.tensor_tensor(hj[:], cj[:], ej[:], sub)
                hnew.append(hj)
                nc.scalar.dma_start(ys[s, ds(j * P, P), :], hj[:])

            # phi4-late: ni(s+1) into fresh B tiles (after t3(s) reads drain)
            if not last:
                nxt_B = new_B()
                for j in range(HK):
                    emit_xni(nxt_B, xt_next, j)

            hcur = hnew
            xt_cur = xt_next
            xt_next = xt_pre
            if not last:
                cur_B = nxt_B


def _prep_inputs(xs, W_ih, W_hh, b, b_n):
    """Build per-core input maps."""
    xs = np.ascontiguousarray(xs, dtype=np.float32)
    w_hh_t = np.ascontiguousarray(W_hh.T, dtype=np.float32)  # (H, G)
    w_ih_t = np.ascontiguousarray(W_ih.T, dtype=np.float32)  # (D, G)
    bcol = np.empty((P, 16), np.float32)
    for g in range(3):
        for j in range(HK):
            bcol[:, g * 4 + j] = b[g * H + j * P : g * H + (j + 1) * P]
    for j in range(HK):
        bcol[:, 12 + j] = b_n[j * P : (j + 1) * P]

    in_maps = []
    for core in range(NCORES):
        xst = np.zeros((S, D, LAN), np.float32)
        for cl in range(CPC):
            c = core * CPC + cl
            lanes = slice(cl * B, (cl + 1) * B)
            if c == 0:
                # time-shifted: slot s == time s for s < L; zeros after
                n = min(L, S)
                xst[0:n, :, lanes] = xs[:, 0:n].transpose(1, 2, 0)
            else:
                t0 = c * L - V
                n = min(S, T - t0)
                xst[0:n, :, lanes] = xs[:, t0 : t0 + n].transpose(1, 2, 0)
        in_maps.append({"xs_t": xst, "w_hh_t": w_hh_t, "w_ih_t": w_ih_t, "bcol": bcol})
    return in_maps


def kernel(xs, W_ih, W_hh, b, b_n):
    xs = np.asarray(xs, dtype=np.float32)
    if "nc" not in _cached:
        _cached["nc"] = build_nc()
    nc = _cached["nc"]
    in_maps = _prep_inputs(xs, W_ih, W_hh, b, b_n)
    res = run_bass_kernel_spmd(nc, in_maps, core_ids=list(range(NCORES)))
    _cached["last_results"] = res
    # assemble (B, T, H)
    ys = np.empty((B, T, H), np.float32)
    for core in range(NCORES):
        out = res.results[core]["ys"]  # (S, H, LAN)
        for cl in range(CPC):
            c = core * CPC + cl
            lanes = slice(cl * B, (cl + 1) * B)
            if c == 0:
                ys[:, 0:L] = out[0:L, :, lanes].transpose(2, 0, 1)
            else:
                ys[:, c * L : (c + 1) * L] = out[V : V + L, :, lanes].transpose(2, 0, 1)
    return ys


# revision 30
# speedup vs baseline: 1.1165x; 1.1165x over previous
"""Trainium2 Bass kernel for nn_CellLayer (GRU over B=16, T=4096, D=256, H=512).

Strategy: chunk-parallel GRU with warmup, in a TRANSPOSED layout:
  - T=4096 split into C=128 chunks of L=32 steps; 16 chunks/core x 16 batch
    = 256 lanes per core, stepped S = L + V slots (V=10 warmup).
  - Chunk 0 is time-shifted (starts exactly at t=0 from h=0, which is exact),
    so no masking is needed anywhere; all S slots are written out and the host
    picks each chunk's valid window.
  - Layout: gates and hidden state are [h-dim (partition), lane (free)].
    Benefits vs the [lane, h-dim] layout:
      * h' is produced directly in the stationary operand layout for the next
        step's W_hh matmul -> no PE transposes, no hT copies.
      * All biases are per-partition -> fused for free into ACT activations
        (sigmoid/tanh bias) and one DVE scalar_tensor_tensor (b_n).
      * With 256 lanes the matmul moving dim is 256 >= 256, so float32r runs
        at 1 cycle/row (full PE speed), same as bf16, with fp32-grade accuracy.
  - Per step: 48 h-side + 24 x-side matmuls of [128c x 128m] x [128c, 256]
    accumulating into 8 PSUM banks (4x prz = r|z pairs, 4x pnn = ni|nh pairs).
    x-matmuls for step s+1 are interleaved mid-stream so the PE never idles.
  - Elementwise gate math split across ACT (r, z, n), DVE (t2, t3, m) and
    GPSIMD/Pool (d, h') so no engine exceeds ~70% and PE stays the bottleneck.
"""

import os
import sys

sys.path.insert(0, "/opt/trn_rl_repo")

import numpy as np

import concourse.bass as bass
import concourse.mybir as mybir
import concourse.tile as tile
from concourse import bacc
from concourse.bass import ds, ts
from concourse.bass_utils import run_bass_kernel_spmd

B, T, D, H = 16, 4096, 256, 512
G = 3 * H
NCORES = 8
CPC = 16  # chunks per core
C = NCORES * CPC  # 128 chunks
L = T // C  # 32 output steps per chunk
V = 10  # warmup steps (fp32 chunked algo: l2 rel err 7.2e-4, max-rel 7.0e-3)
S = L + V  # 42 slots
if os.environ.get("KERNEL_S_OVERRIDE"):  # dev: truncated build for fast iteration
    S = int(os.environ["KERNEL_S_OVERRIDE"])
LAN = CPC * B  # 256 lanes = (chunk_local, batch)
P = 128
DK = D // P  # 2 x-contract chunks
HK = H // P  # 4 h-contract chunks / h subtiles

F32 = mybir.dt.float32
F32R = mybir.dt.float32r

_cached = {}


def build_nc():
    nc = bacc.Bacc(None, target_bir_lowering=False)

    # ---- DRAM I/O (per-core values supplied via in_maps) ----
    # xs_t[s, d, lane]: x for slot s, d-major (zeros for chunk0's tail slots)
    xs_t = nc.declare_dram_parameter("xs_t", [S, D, LAN], F32R, isOutput=False)
    # weights, pre-transposed on host: w_hh_t[h, g], w_ih_t[d, g]
    w_hh_t = nc.declare_dram_parameter("w_hh_t", [H, G], F32R, isOutput=False)
    w_ih_t = nc.declare_dram_parameter("w_ih_t", [D, G], F32R, isOutput=False)
    # bias columns [p, 16]: cols 0-3 b_r_j, 4-7 b_z_j, 8-11 b_in_j, 12-15 b_n_j
    bcol = nc.declare_dram_parameter("bcol", [P, 16], F32, isOutput=False)
    # output: ys[s, h, lane] for ALL slots (host selects valid windows)
    ys = nc.declare_dram_parameter("ys", [S, H, LAN], F32R, isOutput=True)

    with tile.TileContext(nc) as tc:
        _build_body(nc, tc, xs_t, w_hh_t, w_ih_t, bcol, ys)
    nc.compile()
    return nc


def _build_body(nc, tc, xs_t, w_hh_t, w_ih_t, bcol, ys):
    from contextlib import ExitStack

    add = mybir.AluOpType.add
    sub = mybir.AluOpType.subtract
    mult = mybir.AluOpType.mult
    SIG = mybir.ActivationFunctionType.Sigmoid
    TANH = mybir.ActivationFunctionType.Tanh

    def gsl(g, j):  # weight columns of gate g, h-subtile j
        return ds(g * H + j * P, P)

    ctx = ExitStack()
    with ctx:
        const = ctx.enter_context(tc.tile_pool(name="const", bufs=1))
        xpool = ctx.enter_context(tc.tile_pool(name="xpool", bufs=4))
        hpool = ctx.enter_context(tc.tile_pool(name="hpool", bufs=2))
        gates = ctx.enter_context(tc.tile_pool(name="gates", bufs=2))
        psum = ctx.enter_context(tc.tile_pool(name="psum", bufs=1, space="PSUM"))

        # ---- resident constants ----
        # order matters: bc/wih feed step 0 (s0 skips h-matmuls), so the big
        # W_hh transfer goes last, split per contract-chunk, overlapping s=0.
        bc = const.tile([P, 16], F32)
        nc.sync.dma_start(bc[:], bcol[:])
        wih = const.tile([P, DK, G], F32R)
        nc.sync.dma_start(wih[:], w_ih_t.rearrange("(dk p) g -> p dk g", p=P))
        whh = const.tile([P, HK, G], F32R)  # [h%128, h//128, g]

        # ---- h state: 4 subtiles [h%128, lane], ring of 2 each ----
        hcur = []
        for j in range(HK):
            hj = hpool.tile([P, LAN], F32R, name=f"hn{j}")
            nc.vector.memset(hj[:].bitcast(F32), 0.0)
            hcur.append(hj)

# PSUM bank discipline: a matmul with start=True clears the whole bank's
        # has-written bits (data survives, but another group's in-progress
        # accumulation breaks). So within one bank, a group's [first..last]
        # write window must contain no other group's start.
        #   bank A_j = r_j | z_j:  z's group runs strictly after r's stop.
        #   bank B_j = ni_j | nh_j: ni (x-only) closes in step s-1; nh after.

        def new_A():
            return [psum.tile([P, 2 * LAN], F32, name=f"pA{j}") for j in range(HK)]

        def new_B():
            return [psum.tile([P, 2 * LAN], F32, name=f"pB{j}") for j in range(HK)]

        def emit_xr(A, xt, j, s0=False):  # open r window (s0: h==0, close it too)
            for k in range(DK):
                nc.tensor.matmul(A[j][:, 0:LAN], wih[:, k, gsl(0, j)], xt[:, k], start=(k == 0), stop=(s0 and k == DK - 1))

        def emit_xni(B, xt, j):  # ni: x-only, complete group
            for k in range(DK):
                nc.tensor.matmul(B[j][:, 0:LAN], wih[:, k, gsl(2, j)], xt[:, k], start=(k == 0), stop=(k == DK - 1))

        def emit_z_block(A, xt, hsrc, j, s0=False):  # full z group (after r's stop)
            for k in range(DK):
                nc.tensor.matmul(A[j][:, LAN:], wih[:, k, gsl(1, j)], xt[:, k], start=(k == 0), stop=(s0 and k == DK - 1))
            if not s0:
                for k in range(HK):
                    nc.tensor.matmul(A[j][:, LAN:], whh[:, k, gsl(1, j)], hsrc[k][:], start=False, stop=(k == HK - 1))

        # ---- prologue: ni(0), xr(0), xt prefetch ----
        xt_cur = xpool.tile([P, DK, LAN], F32R, name="xt")
        nc.sync.dma_start(xt_cur[:], xs_t[0].rearrange("(dk p) b -> p dk b", p=P))
        xt_next = None
        if S > 1:
            xt_next = xpool.tile([P, DK, LAN], F32R, name="xt")
            nc.sync.dma_start(xt_next[:], xs_t[1].rearrange("(dk p) b -> p dk b", p=P))
        cur_B = new_B()
        for j in range(HK):
            emit_xni(cur_B, xt_cur, j)
            # s=0 skips the h-matmuls entirely (h==0): nh half must read 0
            nc.vector.memset(cur_B[j][:, LAN:], 0.0)
        # big W_hh load queued after everything step 0 needs; first use is s=1
        for k in range(HK):
            nc.sync.dma_start(whh[:, k, :], w_hh_t[ds(k * P, P), :])

        for s in range(S):
            last = s == S - 1
            if not last and s + 2 < S:
                xt_pre = xpool.tile([P, DK, LAN], F32R, name="xt")
                nc.sync.dma_start(xt_pre[:], xs_t[s + 2].rearrange("(dk p) b -> p dk b", p=P))
            else:
                xt_pre = None

            s0 = s == 0  # h==0: h-matmuls skipped; W_hh DMA overlaps step 0
            cur_A = new_A()
            # phi0: open all r windows (x-side, no h dependency)
            for j in range(HK):
                emit_xr(cur_A, xt_cur, j, s0)

            # per-j sets: [hr_j k0..3 | hnh_j k0..3 | z_j block], so each
            # bank's stops stagger early and the gate chain starts ~1.3us in
            # instead of at stream end. Chain tail: c = z*h runs off-chain at
            # z-time; e = (z-1)*n; h' = c - e (one serial stage fewer).
            hnew = []
            for j in range(HK):
                if not s0:
                    for k in range(HK):
                        nc.tensor.matmul(cur_A[j][:, 0:LAN], whh[:, k, gsl(0, j)], hcur[k][:], start=False, stop=(k == HK - 1))
                    for k in range(HK):
                        nc.tensor.matmul(cur_B[j][:, LAN:], whh[:, k, gsl(2, j)], hcur[k][:], start=(k == 0), stop=(k == HK - 1))
                emit_z_block(cur_A, xt_cur, hcur, j, s0)

                rj = gates.tile([P, LAN], F32, name=f"r{j}")
                nc.scalar.activation(rj[:], cur_A[j][:, 0:LAN], SIG, bias=bc[:, ds(j, 1)])
                zj = gates.tile([P, LAN], F32, name=f"z{j}")
                nc.scalar.activation(zj[:], cur_A[j][:, LAN:], SIG, bias=bc[:, ds(4 + j, 1)])
                cj = gates.tile([P, LAN], F32, name=f"c{j}")
                nc.gpsimd.tensor_tensor(cj[:], zj[:], hcur[j][:], mult)
                t2j = gates.tile([P, LAN], F32, name=f"t2{j}")
                nc.vector.scalar_tensor_tensor(t2j[:], cur_B[j][:, LAN:], bc[:, ds(12 + j, 1)], rj[:], add, mult)
                t3j = gates.tile([P, LAN], F32, name=f"t3{j}")
                nc.vector.tensor_tensor(t3j[:], t2j[:], cur_B[j][:, 0:LAN], add)
                nj = gates.tile([P, LAN], F32, name=f"n{j}")
                nc.scalar.activation(nj[:], t3j[:], TANH, bias=bc[:, ds(8 + j, 1)])
                ej = gates.tile([P, LAN], F32, name=f"e{j}")
                nc.vector.scalar_tensor_tensor(ej[:], zj[:], 1.0, nj[:], sub, mult)
                hj = hpool.tile([P, LAN], F32R, name=f"hn{j}")
                heng = nc.gpsimd if j < 2 else nc.vector
                heng.tensor_tensor(hj[:], cj[:], ej[:], sub)
                hnew.append(hj)
                nc.sync.dma_start(ys[s, ds(j * P, P), :], hj[:])

            # phi4-late: ni(s+1) into fresh B tiles (after t3(s) reads drain)
            if not last:
                nxt_B = new_B()
                for j in range(HK):
                    emit_xni(nxt_B, xt_next, j)

            hcur = hnew
            xt_cur = xt_next
            xt_next = xt_pre
            if not last:
                cur_B = nxt_B


def _prep_inputs(xs, W_ih, W_hh, b, b_n):
    """Build per-core input maps."""
    xs = np.ascontiguousarray(xs, dtype=np.float32)
    w_hh_t = np.ascontiguousarray(W_hh.T, dtype=np.float32)  # (H, G)
    w_ih_t = np.ascontiguousarray(W_ih.T, dtype=np.float32)  # (D, G)
    bcol = np.empty((P, 16), np.float32)
    for g in range(3):
        for j in range(HK):
            bcol[:, g * 4 + j] = b[g * H + j * P : g * H + (j + 1) * P]
    for j in range(HK):
        bcol[:, 12 + j] = b_n[j * P : (j + 1) * P]

    in_maps = []
    for core in range(NCORES):
        xst = np.zeros((S, D, LAN), np.float32)
        for cl in range(CPC):
            c = core * CPC + cl
            lanes = slice(cl * B, (cl + 1) * B)
            if c == 0:
                # time-shifted: slot s == time s for s < L; zeros after
                n = min(L, S)
                xst[0:n, :, lanes] = xs[:, 0:n].transpose(1, 2, 0)
            else:
                t0 = c * L - V
                n = min(S, T - t0)
                xst[0:n, :, lanes] = xs[:, t0 : t0 + n].transpose(1, 2, 0)
        in_maps.append({"xs_t": xst, "w_hh_t": w_hh_t, "w_ih_t": w_ih_t, "bcol": bcol})
    return in_maps


def kernel(xs, W_ih, W_hh, b, b_n):
    xs = np.asarray(xs, dtype=np.float32)
    if "nc" not in _cached:
        _cached["nc"] = build_nc()
    nc = _cached["nc"]
    in_maps = _prep_inputs(xs, W_ih, W_hh, b, b_n)
    res = run_bass_kernel_spmd(nc, in_maps, core_ids=list(range(NCORES)))
    _cached["last_results"] = res
    # assemble (B, T, H)
    ys = np.empty((B, T, H), np.float32)
    for core in range(NCORES):
        out = res.results[core]["ys"]  # (S, H, LAN)
        for cl in range(CPC):
            c = core * CPC + cl
            lanes = slice(cl * B, (cl + 1) * B)
            if c == 0:
                ys[:, 0:L] = out[0:L, :, lanes].transpose(2, 0, 1)
            else:
                ys[:, c * L : (c + 1) * L] = out[V : V + L, :, lanes].transpose(2, 0, 1)
    return ys
